# revision 39
# baseline (speedup 1.0000x reference)
"""Trainium2 Bass kernel for a 3-net MLP + masked mean-pooled cross-attention.

B=32 segments data-parallel across 8 NeuronCores (4 per core). The eval-mode
BatchNorm folds into the second MLP layer host-side (y_p = h_p @ A_p + c_p with
h_p the post-LeakyReLU hidden), which lets everything downstream contract
through H=256 instead of D=1024:

  * scores: s = q kT / 32 = h_q (A_q A_kT/32) h_kT + row-const + 1 (x) (rk.h_k)
    with M = A_q A_kT/32 [256,256] and rk = A_k c_q/32 precomputed host-side.
    Row-constant terms are invariant under the row softmax and are dropped;
    the rk term folds into q~ = h_q M + 1 (x) rk as a rank-1 PE update.
  * values: emb = u @ v = (u @ h_v) @ A_v + c_v (sum u = 1), so the [L, D]
    q/k/v tensors are never materialized and the second MLP layer collapses
    to one [256]-vector projection per (direction, segment).
  * max |score| ~ 4, so softmax needs no row-max subtraction; exp directly
    off the score PSUM with accumulated row-sums.
  * key masking is a rank-1 additive -1e6 update (ones (x) mask-row) into the
    score PSUM; exp underflows masked entries to exactly 0.
All matmul operands are bf16 with fp32 PSUM accumulation.
"""

import os
import sys

import numpy as np

for _p in ("/opt/trn_rl_repo", "/root/.axon_site/_ro/trn_rl_repo"):
    if os.path.isdir(_p) and _p not in sys.path:
        sys.path.insert(0, _p)

import ml_dtypes  # noqa: E402

B, LA, LB, D, H, P = 32, 1024, 1024, 1024, 256, 3
BN_EPS = 1e-5
SCALE = 32.0
N_CORES = 8
SEG = B // N_CORES
TOKBLK = 512
NEG = -1e6
DT = D // 128  # 8 d-tiles
HT = H // 128  # 2 h-tiles
NDS = 2 * SEG  # direction-segment slots per core

_CACHE = {}
LAST_RESULTS = None


def _round_up(x, m):
    return (x + m - 1) // m * m


def _chunks(n):
    out, c = [], 0
    while c < n:
        w = min(TOKBLK, n - c)
        out.append((c, w))
        c += w
    return out


def _build_program(sched):
    """sched[(dirn, pos)] = (n_qt, kpad): per segment-position loop structure,
    shared by all cores (SPMD). dirn 0: q from side a, k/v from b."""
    import concourse.bacc as bacc
    import concourse.mybir as mybir
    import concourse.tile as tile

    F32 = mybir.dt.float32
    BF16 = mybir.dt.bfloat16
    AF = mybir.ActivationFunctionType
    ALU = mybir.AluOpType

    nc = bacc.Bacc(
        "TRN2",
        target_bir_lowering=False,
        debug=False,
        enable_asserts=False,
        num_devices=N_CORES,
    )

    # x is pre-transposed host-side to [SEG, DT, 128, LA] so each side-segment
    # is one plain line-rate DMA (DMA_TRANSPOSE is ~2x slower and serializes).
    # x and W1 are fp8 e4m3 (layer-1 runs DoubleRow, 2 d-tiles per matmul);
    # W1/b1 are pre-scaled x8 host-side, undone by the Lrelu activation scale.
    FP8 = mybir.dt.float8e4
    NPAIR = DT // 2
    xa_d = nc.dram_tensor("xa", [SEG, DT, 128, LA], FP8, kind="ExternalInput").ap()
    xb_d = nc.dram_tensor("xb", [SEG, DT, 128, LB], FP8, kind="ExternalInput").ap()
    w1_d = nc.dram_tensor("w1", [128, NPAIR, 2, P * H], FP8, kind="ExternalInput").ap()
    b1_d = nc.dram_tensor("b1", [P, H], BF16, kind="ExternalInput").ap()
    m_d = nc.dram_tensor("m", [H, H], BF16, kind="ExternalInput").ap()
    rk_d = nc.dram_tensor("rk", [H], BF16, kind="ExternalInput").ap()
    av_d = nc.dram_tensor("av", [H, D], BF16, kind="ExternalInput").ap()
    cv_d = nc.dram_tensor("cv", [D], BF16, kind="ExternalInput").ap()
    km_d = nc.dram_tensor("km", [2, SEG, LA], BF16, kind="ExternalInput").ap()
    wb_d = nc.dram_tensor("wb", [2, SEG, LA], F32, kind="ExternalInput").ap()
    o_d = nc.dram_tensor("o", [2, SEG, D], F32, kind="ExternalOutput").ap()

    # per-position padded side lengths (side a / side b tokens needed)
    lpad = {}
    for pos in range(SEG):
        lpad[("a", pos)] = sched[(1, pos)][1]  # a is key side of dirn 1
        lpad[("b", pos)] = sched[(0, pos)][1]

    with tile.TileContext(nc) as tc:
        with (
            tc.tile_pool(name="consts", bufs=1) as consts,
            tc.tile_pool(name="xt", bufs=2) as xtp,
            tc.tile_pool(name="hp", bufs=2) as hpp,
            tc.tile_pool(name="qt", bufs=2) as qtp,
            tc.tile_pool(name="epool", bufs=9) as epool,
            tc.tile_pool(name="stats", bufs=10) as stats,
            tc.tile_pool(name="ubc", bufs=2) as ubcp,
            tc.tile_pool(name="scratch", bufs=2) as scrp,
            tc.tile_pool(name="tpool", bufs=1) as tpool,
            tc.tile_pool(name="opool", bufs=1) as opool,
            tc.tile_pool(name="psA", bufs=2, space="PSUM") as psA,
            tc.tile_pool(name="psS", bufs=2, space="PSUM") as psS,
            tc.tile_pool(name="psU", bufs=2, space="PSUM") as psU,
            tc.tile_pool(name="dramp", bufs=2, space="DRAM") as dramp,
        ):
            # ---- constants ----
            w1_sb = consts.tile([128, NPAIR, 2, P * H], FP8, name="w1sb")
            nc.sync.dma_start(out=w1_sb, in_=w1_d)
            b1_sb = consts.tile([1, P * H], BF16)
            nc.sync.dma_start(out=b1_sb, in_=b1_d.rearrange("p h -> (p h)").unsqueeze(0))
            ones_sb = consts.tile([1, TOKBLK], BF16)
            nc.vector.memset(ones_sb, 1.0)

            def load_xt(seg, side, x2d, xt):
                lp = lpad[(side, seg)]
                nc.sync.dma_start(
                    out=xt[:, :, :lp],
                    in_=x2d[seg].transpose([1, 0, 2])[:, :, :lp],
                )

            xt_tiles = {}
            for seg in range(SEG):
                xt_tiles[seg] = (
                    xtp.tile([128, DT, 1024], FP8, tag="xta", name=f"xta{seg}"),
                    xtp.tile([128, DT, 1024], FP8, tag="xtb", name=f"xtb{seg}"),
                )
            # seg 0's inputs ahead of the remaining consts, split per token
            # chunk: PE's first MLP group only needs w1/b1 + the first chunk.
            for side_i, (side, x2d) in enumerate((("a", xa_d), ("b", xb_d))):
                lp = lpad[(side, 0)]
                for c0, cw in _chunks(lp):
                    nc.sync.dma_start(
                        out=xt_tiles[0][side_i][:, :, c0 : c0 + cw],
                        in_=x2d[0].transpose([1, 0, 2])[:, :, c0 : c0 + cw],
                    )

            m_sb = consts.tile([128, HT * H], BF16)
            for hi in range(HT):
                nc.sync.dma_start(
                    out=m_sb[:, hi * H : (hi + 1) * H],
                    in_=m_d[hi * 128 : (hi + 1) * 128, :],
                )
            rk_sb = consts.tile([1, H], BF16)
            nc.sync.dma_start(out=rk_sb, in_=rk_d.unsqueeze(0))
            av_sb = consts.tile([128, HT * D], BF16)
            for hi in range(HT):
                nc.sync.dma_start(
                    out=av_sb[:, hi * D : (hi + 1) * D],
                    in_=av_d[hi * 128 : (hi + 1) * 128, :],
                )
            cv_sb = consts.tile([1, D], BF16)
            nc.sync.dma_start(out=cv_sb, in_=cv_d.unsqueeze(0))
            km_sb = consts.tile([1, 2 * SEG * LA], BF16)
            nc.sync.dma_start(out=km_sb, in_=km_d.rearrange("a s l -> (a s l)").unsqueeze(0))
            wb_sb = consts.tile([128, 2 * SEG * 8], F32)
            nc.sync.dma_start(out=wb_sb, in_=wb_d.rearrange("a s (t p) -> p (a s t)", p=128))
            t_f32 = [tpool.tile([128, NDS], F32, name=f"tf{ht}") for ht in range(HT)]

            # h output tile + group index per MLP group g = net*2+ht:
            # q-net (0,1) and v-net (4,5) stay bf16; k-net (2,3) goes fp8 so the
            # score matmuls can run DoubleRow.
            def h_slot(g):
                return (0, g) if g < 2 else ((1, g - 2) if g < 4 else (0, g - 2))

            def mlp_units(seg, side, xt, hb_sb, h8_sb):
                """Generator: one yield per (chunk, group) PSUM unit, so MLP
                work can be interleaved as PE filler inside attention loops.
                hb_sb: [128, 4, 1024] bf16 (q0,q1,v0,v1); h8_sb: [128, 2, 1024] fp8."""
                lp = lpad[(side, seg)]
                for c0, cw in _chunks(lp):
                    for g in range(P * HT):
                        which, gi = h_slot(g)
                        h_sb = hb_sb if which == 0 else h8_sb
                        hp = psA.tile([128, TOKBLK], F32, tag="ps_a", name=f"hp{seg}{side}{g}{c0}")
                        for q in range(NPAIR):
                            nc.tensor.matmul(
                                hp[:, :cw],
                                w1_sb[:, q, :, g * 128 : (g + 1) * 128],
                                xt[:, 2 * q : 2 * q + 2, c0 : c0 + cw],
                                start=(q == 0),
                                stop=False,
                                perf_mode=mybir.MatmulPerfMode.DoubleRow,
                            )
                        nc.tensor.matmul(
                            hp[:, :cw],
                            b1_sb[:, g * 128 : (g + 1) * 128],
                            ones_sb[:, :cw],
                            start=False,
                            stop=True,
                        )
                        # LeakyReLU (slope 0.01 per PWP table); scale undoes the
                        # x8 pre-scaling of W1/b1. PSUM f32 -> SBUF bf16/fp8.
                        nc.scalar.activation(
                            out=h_sb[:, gi, c0 : c0 + cw], in_=hp[:, :cw], func=AF.Lrelu,
                            scale=0.125,
                        )
                        yield

            def drain(gen):
                if gen is not None:
                    for _ in gen:
                        pass

            def attention(seg, dirn, hb_q, hb_k, h8_k, last=False, filler=None):
                """hb_q[:, 0:2]: q-net of the query side (bf16). h8_k: k-net of
                the key side (fp8, DoubleRow pair); hb_k[:, 2:4]: v-net (bf16)."""
                n_qt, kpad = sched[(dirn, seg)]
                lq = n_qt * 128
                kch = _chunks(kpad)
                bd = dirn * SEG + seg

                # q~ = h_q M + 1 (x) rk, feature-major, cast fp8 (scores carry
                # the 32x score scale; it is undone by the exp activation scale)
                qt_sb = qtp.tile([128, HT, 1024], FP8, tag="qt", name=f"qt{bd}")
                for ho in range(HT):
                    for c0, cw in _chunks(lq):
                        qp = psA.tile([128, TOKBLK], F32, tag="ps_a", name=f"qp{bd}{ho}{c0}")
                        for hi in range(HT):
                            nc.tensor.matmul(
                                qp[:, :cw],
                                m_sb[:, hi * H + ho * 128 : hi * H + ho * 128 + 128],
                                hb_q[:, hi, c0 : c0 + cw],
                                start=(hi == 0),
                                stop=False,
                            )
                        nc.tensor.matmul(
                            qp[:, :cw],
                            rk_sb[:, ho * 128 : (ho + 1) * 128],
                            ones_sb[:, :cw],
                            start=False,
                            stop=True,
                        )
                        nc.vector.tensor_copy(out=qt_sb[:, ho, c0 : c0 + cw], in_=qp[:, :cw])

                u_ps = [
                    psU.tile([1, TOKBLK], F32, tag="ps_u", name=f"u{bd}_{ci}")
                    for ci in range(len(kch))
                ]

                # Score loop: PE streams all qt back-to-back (scores have no
                # dependency on the softmax stats, so no PE stalls and HAM
                # stays warm). exp/recip trail on ACT/DVE; the u matmuls run
                # as one dense burst afterwards from the retained e tiles.
                e_tiles, w_tiles = [], []
                for qt in range(n_qt):
                    sp = psS.tile([128, 1024], F32, tag="ps_s", name=f"s{bd}_{qt}")
                    for c0, cw in kch:
                        nc.tensor.matmul(
                            sp[:, c0 : c0 + cw],
                            qt_sb[:, :, qt * 128 : (qt + 1) * 128],
                            h8_k[:, :, c0 : c0 + cw],
                            start=True,
                            stop=False,
                            perf_mode=mybir.MatmulPerfMode.DoubleRow,
                        )
                        nc.tensor.matmul(
                            sp[:, c0 : c0 + cw],
                            ones_sb[:, :128],
                            km_sb[:, bd * LA + c0 : bd * LA + c0 + cw],
                            start=False,
                            stop=True,
                        )
                    e = epool.tile([128, 1024], BF16, tag="e", name=f"e{bd}_{qt}")
                    z = stats.tile([128, 1], F32, tag="z", name=f"z{bd}_{qt}")
                    nc.scalar.activation(
                        out=e[:, :kpad], in_=sp[:, :kpad], func=AF.Exp,
                        scale=1.0 / SCALE, accum_out=z,
                    )
                    rz = stats.tile([128, 1], F32, tag="rz", name=f"rz{bd}_{qt}")
                    nc.vector.reciprocal(out=rz, in_=z)
                    w = stats.tile([128, 1], BF16, tag="w", name=f"w{bd}_{qt}")
                    nc.vector.tensor_tensor(
                        out=w, in0=wb_sb[:, bd * 8 + qt : bd * 8 + qt + 1], in1=rz,
                        op=ALU.mult,
                    )
                    e_tiles.append(e)
                    w_tiles.append(w)
                    # keep PE dense while ACT catches up on the exps: emit one
                    # next-segment MLP unit between score tiles
                    if filler is not None:
                        next(filler, None)
                for qt in range(n_qt):
                    for ci, (c0, cw) in enumerate(kch):
                        nc.tensor.matmul(
                            u_ps[ci][:, :cw], w_tiles[qt], e_tiles[qt][:, c0 : c0 + cw],
                            start=(qt == 0), stop=(qt == n_qt - 1),
                        )

                # u -> SBUF, broadcast to 128 partitions, t = u . h_v via DVE.
                # Mid-kernel dirs use a DRAM-roundtrip broadcast (no PSUM slot
                # contention); the last dir broadcasts via a rank-1 PE matmul
                # into the now-free score PSUM to cut the serial tail.
                u_sb = stats.tile([1, 1024], BF16, tag="u_sb", name=f"usb{bd}")
                for ci, (c0, cw) in enumerate(kch):
                    nc.vector.tensor_copy(out=u_sb[:, c0 : c0 + cw], in_=u_ps[ci][:, :cw])
                if last:
                    u_bc = psS.tile([128, 1024], F32, tag="ps_s", name=f"ubc{bd}")
                    for c0, cw in kch:
                        nc.tensor.matmul(
                            u_bc[:, c0 : c0 + cw],
                            ones_sb[:, :128],
                            u_sb[:, c0 : c0 + cw],
                            start=True,
                            stop=True,
                        )
                else:
                    u_dr = dramp.tile([1, kpad], BF16, tag="u_dr", name=f"udr{bd}")
                    nc.sync.dma_start(out=u_dr, in_=u_sb[:, :kpad])
                    u_bc = ubcp.tile([128, 1024], BF16, tag="u_bc", name=f"ubc{bd}")
                    nc.sync.dma_start(
                        out=u_bc[:, :kpad], in_=u_dr[0].partition_broadcast(128)
                    )
                for ht in range(HT):
                    prod = scrp.tile([128, 1024], BF16, tag="prod", name=f"prod{bd}_{ht}")
                    nc.vector.scalar_tensor_tensor(
                        out=prod[:, :kpad], in0=hb_k[:, 2 + ht, :kpad], scalar=1.0,
                        in1=u_bc[:, :kpad], op0=ALU.mult, op1=ALU.mult,
                        accum_out=t_f32[ht][:, bd : bd + 1],
                    )

            h_tiles = {}
            for seg in range(SEG):
                h_tiles[seg] = (
                    hpp.tile([128, 4, 1024], BF16, tag="hba", name=f"hba{seg}"),
                    hpp.tile([128, 4, 1024], BF16, tag="hbb", name=f"hbb{seg}"),
                    hpp.tile([128, 2, 1024], FP8, tag="h8a", name=f"h8a{seg}"),
                    hpp.tile([128, 2, 1024], FP8, tag="h8b", name=f"h8b{seg}"),
                )

            def mlp_gens(seg):
                hb_a, hb_b, h8_a, h8_b = h_tiles[seg]
                return (
                    mlp_units(seg, "a", xt_tiles[seg][0], hb_a, h8_a),
                    mlp_units(seg, "b", xt_tiles[seg][1], hb_b, h8_b),
                )

            # seg 0's MLP runs inline; each later segment's MLP is emitted as
            # filler inside the previous segment's attention score loops.
            g_a, g_b = mlp_gens(0)
            drain(g_a)
            drain(g_b)
            for seg in range(SEG):
                hb_a, hb_b, h8_a, h8_b = h_tiles[seg]
                nxt_a = nxt_b = None
                if seg + 1 < SEG:
                    load_xt(seg + 1, "a", xa_d, xt_tiles[seg + 1][0])
                    load_xt(seg + 1, "b", xb_d, xt_tiles[seg + 1][1])
                    nxt_a, nxt_b = mlp_gens(seg + 1)
                attention(seg, 0, hb_a, hb_b, h8_b, filler=nxt_a)
                attention(seg, 1, hb_b, hb_a, h8_a, last=(seg == SEG - 1), filler=nxt_b)
                drain(nxt_a)
                drain(nxt_b)

            # final projection, transposed so DRAM writes are contiguous:
            # o[ds, d] = sum_hi T[hi, ds] A_v[hi, d] + c_v[d]  (ds on partitions)
            t_bf = [tpool.tile([128, NDS], BF16, name=f"tb{ht}") for ht in range(HT)]
            for ht in range(HT):
                nc.vector.tensor_copy(out=t_bf[ht], in_=t_f32[ht])
            o_ps = psS.tile([128, D], F32, tag="ps_s", name="ops")
            for c0, cw in _chunks(D):
                for hi in range(HT):
                    nc.tensor.matmul(
                        o_ps[:NDS, c0 : c0 + cw],
                        t_bf[hi],
                        av_sb[:, hi * D + c0 : hi * D + c0 + cw],
                        start=(hi == 0),
                        stop=False,
                    )
                nc.tensor.matmul(
                    o_ps[:NDS, c0 : c0 + cw],
                    ones_sb[:, :NDS],
                    cv_sb[:, c0 : c0 + cw],
                    start=False,
                    stop=True,
                )
            o_sb = opool.tile([NDS, D], F32)
            nc.vector.tensor_copy(out=o_sb, in_=o_ps[:NDS, :])
            nc.sync.dma_start(out=o_d.rearrange("a s d -> (a s) d"), in_=o_sb)

    nc.compile()
    return nc


def _preprocess(inputs):
    """Host-side folding + sharding. Returns (sched, in_maps, perm)."""
    a = np.asarray(inputs["a"], dtype=np.float32)
    b = np.asarray(inputs["b"], dtype=np.float32)
    W1 = np.asarray(inputs["W1"], dtype=np.float32)
    b1 = np.asarray(inputs["b1"], dtype=np.float32)
    g = np.asarray(inputs["g"], dtype=np.float32)
    bt = np.asarray(inputs["bt"], dtype=np.float32)
    rm = np.asarray(inputs["rm"], dtype=np.float32)
    rv = np.asarray(inputs["rv"], dtype=np.float32)
    W2 = np.asarray(inputs["W2"], dtype=np.float32)
    b2 = np.asarray(inputs["b2"], dtype=np.float32)
    len_a = np.asarray(inputs["len_a"], dtype=np.int64)
    len_b = np.asarray(inputs["len_b"], dtype=np.int64)

    alpha = g / np.sqrt(rv + BN_EPS)
    beta = bt - rm * alpha
    A = W2 * alpha[:, :, None]  # [3, H, D]
    c = np.einsum("ph,phd->pd", beta, W2) + b2  # [3, D]
    # score scale (1/32) is NOT folded here: q~ is cast to fp8 on device and
    # the x32-larger values sit in e4m3's normal range; exp applies 1/SCALE.
    M = A[0] @ A[1].T  # [H, H]
    rk = A[1] @ c[0]  # [H]

    bf16 = ml_dtypes.bfloat16
    f8 = ml_dtypes.float8_e4m3
    # W1/b1 x8 keeps the small layer-1 weights out of fp8's subnormal range;
    # the kernel's Lrelu activation scale (1/8) undoes it after the matmul.
    # DoubleRow stationary layout: [p, pair, member, g*128+h] with
    # d = (2*pair+member)*128 + p.
    w1p = np.ascontiguousarray(
        (8.0 * W1).reshape(P, DT // 2, 2, 128, H // 128, 128)
        .transpose(3, 1, 2, 0, 4, 5)
        .reshape(128, DT // 2, 2, P * H)
        .astype(f8)
    )
    b1_bf = np.ascontiguousarray((8.0 * b1).astype(bf16))
    m_bf = np.ascontiguousarray(M.astype(bf16))
    rk_bf = np.ascontiguousarray(rk.astype(bf16))
    av_bf = np.ascontiguousarray(A[2].astype(bf16))
    cv_bf = np.ascontiguousarray(c[2].astype(bf16))

    # Segment -> (core, position): sort by score cost so each position's
    # cross-core max (which fixes the SPMD loop bounds) is small.
    order = np.argsort(-(len_a * len_b), kind="stable")
    perm = [[int(order[pos * N_CORES + cc]) for pos in range(SEG)] for cc in range(N_CORES)]

    sched = {}
    for pos in range(SEG):
        segs = [perm[cc][pos] for cc in range(N_CORES)]
        for dirn in range(2):
            lq = max((len_a if dirn == 0 else len_b)[s] for s in segs)
            lk = max((len_b if dirn == 0 else len_a)[s] for s in segs)
            sched[(dirn, pos)] = (
                _round_up(int(lq), 128) // 128,
                _round_up(int(lk), 128),
            )

    iota = np.arange(LA)
    in_maps = []
    for cc in range(N_CORES):
        segs = perm[cc]
        # [SEG, DT, 128, LA] feature-major fp8 layout (see _build_program)
        xa = np.ascontiguousarray(
            a[segs].reshape(SEG, LA, DT, 128).transpose(0, 2, 3, 1).astype(f8)
        )
        xb = np.ascontiguousarray(
            b[segs].reshape(SEG, LB, DT, 128).transpose(0, 2, 3, 1).astype(f8)
        )
        km = np.zeros((2, SEG, LA), dtype=np.float32)
        wb = np.zeros((2, SEG, LA), dtype=np.float32)
        for pos, s in enumerate(segs):
            for dirn in range(2):
                lq = int((len_a if dirn == 0 else len_b)[s])
                lk = int((len_b if dirn == 0 else len_a)[s])
                km[dirn, pos, :] = np.where(iota < lk, 0.0, NEG * SCALE)
                wb[dirn, pos, :] = np.where(iota < lq, 1.0 / lq, 0.0)
        in_maps.append(
            {
                "xa": xa,
                "xb": xb,
                "w1": w1p,
                "b1": b1_bf,
                "m": m_bf,
                "rk": rk_bf,
                "av": av_bf,
                "cv": cv_bf,
                "km": np.ascontiguousarray(km.astype(bf16)),
                "wb": np.ascontiguousarray(wb),
            }
        )
    return sched, in_maps, perm


def kernel(**inputs):
    global LAST_RESULTS
    from concourse.bass_utils import run_bass_kernel_spmd

    sched, in_maps, perm = _preprocess(inputs)
    key = tuple(sorted(sched.items()))
    if key not in _CACHE:
        _CACHE[key] = _build_program(sched)
    nc = _CACHE[key]

    res = run_bass_kernel_spmd(nc, in_maps, list(range(N_CORES)))
    LAST_RESULTS = res

    out = np.zeros((2, B, D), dtype=np.float32)
    for cc in range(N_CORES):
        o = res.results[cc]["o"]  # [2, SEG, D]
        for pos, s in enumerate(perm[cc]):
            out[0, s] = o[0, pos]
            out[1, s] = o[1, pos]
    return out


# revision 43
# speedup vs baseline: 1.1219x; 1.1219x over previous
"""Trainium2 Bass kernel for a 3-net MLP + masked mean-pooled cross-attention.

B=32 segments data-parallel across 8 NeuronCores (4 per core). The eval-mode
BatchNorm folds into the second MLP layer host-side (y_p = h_p @ A_p + c_p with
h_p the post-LeakyReLU hidden), which lets everything downstream contract
through H=256 instead of D=1024:

  * scores: s = q kT / 32 = h_q (A_q A_kT/32) h_kT + row-const + 1 (x) (rk.h_k)
    with M = A_q A_kT/32 [256,256] and rk = A_k c_q/32 precomputed host-side.
    Row-constant terms are invariant under the row softmax and are dropped;
    the rk term folds into q~ = h_q M + 1 (x) rk as a rank-1 PE update.
  * values: emb = u @ v = (u @ h_v) @ A_v + c_v (sum u = 1), so the [L, D]
    q/k/v tensors are never materialized and the second MLP layer collapses
    to one [256]-vector projection per (direction, segment).
  * max |score| ~ 4, so softmax needs no row-max subtraction; exp directly
    off the score PSUM with accumulated row-sums.
  * key masking is a rank-1 additive -1e6 update (ones (x) mask-row) into the
    score PSUM; exp underflows masked entries to exactly 0.
All matmul operands are bf16 with fp32 PSUM accumulation.
"""

import os
import sys

import numpy as np

for _p in ("/opt/trn_rl_repo", "/root/.axon_site/_ro/trn_rl_repo"):
    if os.path.isdir(_p) and _p not in sys.path:
        sys.path.insert(0, _p)

import ml_dtypes  # noqa: E402

B, LA, LB, D, H, P = 32, 1024, 1024, 1024, 256, 3
BN_EPS = 1e-5
SCALE = 32.0
N_CORES = 8
SEG = B // N_CORES
TOKBLK = 512
NEG = -1e6
DT = D // 128  # 8 d-tiles
HT = H // 128  # 2 h-tiles
NDS = 2 * SEG  # direction-segment slots per core

_CACHE = {}
LAST_RESULTS = None


def _round_up(x, m):
    return (x + m - 1) // m * m


def _chunks(n):
    out, c = [], 0
    while c < n:
        w = min(TOKBLK, n - c)
        out.append((c, w))
        c += w
    return out


def _build_program(sched):
    """sched[(dirn, pos)] = (n_qt, kpad): per segment-position loop structure,
    shared by all cores (SPMD). dirn 0: q from side a, k/v from b."""
    import concourse.bacc as bacc
    import concourse.mybir as mybir
    import concourse.tile as tile

    F32 = mybir.dt.float32
    BF16 = mybir.dt.bfloat16
    AF = mybir.ActivationFunctionType
    ALU = mybir.AluOpType

    nc = bacc.Bacc(
        "TRN2",
        target_bir_lowering=False,
        debug=False,
        enable_asserts=False,
        num_devices=N_CORES,
    )

    # x is pre-transposed host-side to [SEG, DT, 128, LA] so each side-segment
    # is one plain line-rate DMA (DMA_TRANSPOSE is ~2x slower and serializes).
    # x and W1 are fp8 e4m3 (layer-1 runs DoubleRow, 2 d-tiles per matmul);
    # W1/b1 are pre-scaled x8 host-side, undone by the Lrelu activation scale.
    FP8 = mybir.dt.float8e4
    NPAIR = DT // 2
    xa_d = nc.dram_tensor("xa", [SEG, DT, 128, LA], FP8, kind="ExternalInput").ap()
    xb_d = nc.dram_tensor("xb", [SEG, DT, 128, LB], FP8, kind="ExternalInput").ap()
    w1_d = nc.dram_tensor("w1", [128, NPAIR, 2, P * H], FP8, kind="ExternalInput").ap()
    b1_d = nc.dram_tensor("b1", [P, H], BF16, kind="ExternalInput").ap()
    m_d = nc.dram_tensor("m", [H, H], BF16, kind="ExternalInput").ap()
    rk_d = nc.dram_tensor("rk", [H], BF16, kind="ExternalInput").ap()
    av_d = nc.dram_tensor("av", [H, D], BF16, kind="ExternalInput").ap()
    cv_d = nc.dram_tensor("cv", [D], BF16, kind="ExternalInput").ap()
    km_d = nc.dram_tensor("km", [2, SEG, LA], BF16, kind="ExternalInput").ap()
    wb_d = nc.dram_tensor("wb", [2, SEG, LA], F32, kind="ExternalInput").ap()
    o_d = nc.dram_tensor("o", [2, SEG, D], F32, kind="ExternalOutput").ap()

    # per-position padded side lengths (side a / side b tokens needed)
    lpad = {}
    for pos in range(SEG):
        lpad[("a", pos)] = sched[(1, pos)][1]  # a is key side of dirn 1
        lpad[("b", pos)] = sched[(0, pos)][1]

    with tile.TileContext(nc) as tc:
        with (
            tc.tile_pool(name="consts", bufs=1) as consts,
            tc.tile_pool(name="xt", bufs=2) as xtp,
            tc.tile_pool(name="hp", bufs=2) as hpp,
            tc.tile_pool(name="qt", bufs=2) as qtp,
            tc.tile_pool(name="epool", bufs=9) as epool,
            tc.tile_pool(name="stats", bufs=10) as stats,
            tc.tile_pool(name="ubc", bufs=2) as ubcp,
            tc.tile_pool(name="scratch", bufs=2) as scrp,
            tc.tile_pool(name="tpool", bufs=1) as tpool,
            tc.tile_pool(name="opool", bufs=1) as opool,
            tc.tile_pool(name="psA", bufs=2, space="PSUM") as psA,
            tc.tile_pool(name="psS", bufs=2, space="PSUM") as psS,
            tc.tile_pool(name="psU", bufs=2, space="PSUM") as psU,
            tc.tile_pool(name="dramp", bufs=2, space="DRAM") as dramp,
        ):
            # ---- constants ----
            w1_sb = consts.tile([128, NPAIR, 2, P * H], FP8, name="w1sb")
            nc.sync.dma_start(out=w1_sb, in_=w1_d)
            b1_sb = consts.tile([1, P * H], BF16)
            nc.sync.dma_start(out=b1_sb, in_=b1_d.rearrange("p h -> (p h)").unsqueeze(0))
            ones_sb = consts.tile([1, TOKBLK], BF16)
            nc.vector.memset(ones_sb, 1.0)

            def load_xt(seg, side, x2d, xt):
                lp = lpad[(side, seg)]
                nc.sync.dma_start(
                    out=xt[:, :, :lp],
                    in_=x2d[seg].transpose([1, 0, 2])[:, :, :lp],
                )

            xt_tiles = {}
            for seg in range(SEG):
                xt_tiles[seg] = (
                    xtp.tile([128, DT, 1024], FP8, tag="xta", name=f"xta{seg}"),
                    xtp.tile([128, DT, 1024], FP8, tag="xtb", name=f"xtb{seg}"),
                )
            # seg 0's inputs ahead of the remaining consts, split per token
            # chunk: PE's first MLP group only needs w1/b1 + the first chunk.
            for side_i, (side, x2d) in enumerate((("a", xa_d), ("b", xb_d))):
                lp = lpad[(side, 0)]
                for c0, cw in _chunks(lp):
                    nc.sync.dma_start(
                        out=xt_tiles[0][side_i][:, :, c0 : c0 + cw],
                        in_=x2d[0].transpose([1, 0, 2])[:, :, c0 : c0 + cw],
                    )

            m_sb = consts.tile([128, HT * H], BF16)
            for hi in range(HT):
                nc.sync.dma_start(
                    out=m_sb[:, hi * H : (hi + 1) * H],
                    in_=m_d[hi * 128 : (hi + 1) * 128, :],
                )
            rk_sb = consts.tile([1, H], BF16)
            nc.sync.dma_start(out=rk_sb, in_=rk_d.unsqueeze(0))
            av_sb = consts.tile([128, HT * D], BF16)
            for hi in range(HT):
                nc.sync.dma_start(
                    out=av_sb[:, hi * D : (hi + 1) * D],
                    in_=av_d[hi * 128 : (hi + 1) * 128, :],
                )
            cv_sb = consts.tile([1, D], BF16)
            nc.sync.dma_start(out=cv_sb, in_=cv_d.unsqueeze(0))
            km_sb = consts.tile([1, 2 * SEG * LA], BF16)
            nc.sync.dma_start(out=km_sb, in_=km_d.rearrange("a s l -> (a s l)").unsqueeze(0))
            wb_sb = consts.tile([128, 2 * SEG * 8], F32)
            nc.sync.dma_start(out=wb_sb, in_=wb_d.rearrange("a s (t p) -> p (a s t)", p=128))
            t_f32 = [tpool.tile([128, NDS], F32, name=f"tf{ht}") for ht in range(HT)]

            # h output tile + group index per MLP group g = net*2+ht:
            # q-net (0,1) and v-net (4,5) stay bf16; k-net (2,3) goes fp8 so the
            # score matmuls can run DoubleRow.
            def h_slot(g):
                return (0, g) if g < 2 else ((1, g - 2) if g < 4 else (0, g - 2))

            def mlp_units(seg, side, xt, hb_sb, h8_sb, act="act"):
                """Generator: one yield per (chunk, group) PSUM unit, so MLP
                work can be interleaved as PE filler inside attention loops.
                act="act": ACT Lrelu (PSUM->SBUF, 1 op). act="dve": DVE
                copy+max pair, keeping ACT free for the attention exps.
                hb_sb: [128, 4, 1024] bf16 (q0,q1,v0,v1); h8_sb: [128, 2, 1024] fp8."""
                lp = lpad[(side, seg)]
                for c0, cw in _chunks(lp):
                    for g in range(P * HT):
                        which, gi = h_slot(g)
                        h_sb = hb_sb if which == 0 else h8_sb
                        hp = psA.tile([128, TOKBLK], F32, tag="ps_a", name=f"hp{seg}{side}{g}{c0}")
                        for q in range(NPAIR):
                            nc.tensor.matmul(
                                hp[:, :cw],
                                w1_sb[:, q, :, g * 128 : (g + 1) * 128],
                                xt[:, 2 * q : 2 * q + 2, c0 : c0 + cw],
                                start=(q == 0),
                                stop=False,
                                perf_mode=mybir.MatmulPerfMode.DoubleRow,
                            )
                        nc.tensor.matmul(
                            hp[:, :cw],
                            b1_sb[:, g * 128 : (g + 1) * 128],
                            ones_sb[:, :cw],
                            start=False,
                            stop=True,
                        )
                        # LeakyReLU (slope 0.01); the 1/8 scale undoes the x8
                        # pre-scaling of W1/b1. PSUM f32 -> SBUF bf16/fp8.
                        if act == "act":
                            nc.scalar.activation(
                                out=h_sb[:, gi, c0 : c0 + cw], in_=hp[:, :cw],
                                func=AF.Lrelu, scale=0.125,
                            )
                        else:
                            lr = scrp.tile([128, TOKBLK], BF16, tag="lr", name=f"lr{seg}{side}{g}{c0}")
                            nc.vector.tensor_scalar_mul(
                                out=lr[:, :cw], in0=hp[:, :cw], scalar1=0.125,
                            )
                            nc.vector.scalar_tensor_tensor(
                                out=h_sb[:, gi, c0 : c0 + cw], in0=lr[:, :cw],
                                scalar=0.01, in1=lr[:, :cw], op0=ALU.mult, op1=ALU.max,
                            )
                        yield

            def drain(gen):
                if gen is not None:
                    for _ in gen:
                        pass

            def attention(seg, dirn, hb_q, hb_k, h8_k, last=False, filler=None):
                """hb_q[:, 0:2]: q-net of the query side (bf16). h8_k: k-net of
                the key side (fp8, DoubleRow pair); hb_k[:, 2:4]: v-net (bf16)."""
                n_qt, kpad = sched[(dirn, seg)]
                lq = n_qt * 128
                kch = _chunks(kpad)
                bd = dirn * SEG + seg

                # q~ = h_q M + 1 (x) rk, feature-major, cast fp8 (scores carry
                # the 32x score scale; it is undone by the exp activation scale)
                qt_sb = qtp.tile([128, HT, 1024], FP8, tag="qt", name=f"qt{bd}")
                for ho in range(HT):
                    for c0, cw in _chunks(lq):
                        qp = psA.tile([128, TOKBLK], F32, tag="ps_a", name=f"qp{bd}{ho}{c0}")
                        for hi in range(HT):
                            nc.tensor.matmul(
                                qp[:, :cw],
                                m_sb[:, hi * H + ho * 128 : hi * H + ho * 128 + 128],
                                hb_q[:, hi, c0 : c0 + cw],
                                start=(hi == 0),
                                stop=False,
                            )
                        nc.tensor.matmul(
                            qp[:, :cw],
                            rk_sb[:, ho * 128 : (ho + 1) * 128],
                            ones_sb[:, :cw],
                            start=False,
                            stop=True,
                        )
                        nc.vector.tensor_copy(out=qt_sb[:, ho, c0 : c0 + cw], in_=qp[:, :cw])

                u_ps = [
                    psU.tile([1, TOKBLK], F32, tag="ps_u", name=f"u{bd}_{ci}")
                    for ci in range(len(kch))
                ]

                # Score loop: PE streams all qt back-to-back (scores have no
                # dependency on the softmax stats, so no PE stalls and HAM
                # stays warm). exp/recip trail on ACT/DVE; the u matmuls run
                # as one dense burst afterwards from the retained e tiles.
                e_tiles, w_tiles = [], []
                for qt in range(n_qt):
                    sp = psS.tile([128, 1024], F32, tag="ps_s", name=f"s{bd}_{qt}")
                    for c0, cw in kch:
                        nc.tensor.matmul(
                            sp[:, c0 : c0 + cw],
                            qt_sb[:, :, qt * 128 : (qt + 1) * 128],
                            h8_k[:, :, c0 : c0 + cw],
                            start=True,
                            stop=False,
                            perf_mode=mybir.MatmulPerfMode.DoubleRow,
                        )
                        nc.tensor.matmul(
                            sp[:, c0 : c0 + cw],
                            ones_sb[:, :128],
                            km_sb[:, bd * LA + c0 : bd * LA + c0 + cw],
                            start=False,
                            stop=True,
                        )
                    e = epool.tile([128, 1024], BF16, tag="e", name=f"e{bd}_{qt}")
                    z = stats.tile([128, 1], F32, tag="z", name=f"z{bd}_{qt}")
                    nc.scalar.activation(
                        out=e[:, :kpad], in_=sp[:, :kpad], func=AF.Exp,
                        scale=1.0 / SCALE, accum_out=z,
                    )
                    rz = stats.tile([128, 1], F32, tag="rz", name=f"rz{bd}_{qt}")
                    nc.vector.reciprocal(out=rz, in_=z)
                    w = stats.tile([128, 1], BF16, tag="w", name=f"w{bd}_{qt}")
                    nc.vector.tensor_tensor(
                        out=w, in0=wb_sb[:, bd * 8 + qt : bd * 8 + qt + 1], in1=rz,
                        op=ALU.mult,
                    )
                    e_tiles.append(e)
                    w_tiles.append(w)
                    # keep PE dense while ACT catches up on the exps: emit one
                    # next-segment MLP unit between score tiles
                    if filler is not None:
                        next(filler, None)
                for qt in range(n_qt):
                    for ci, (c0, cw) in enumerate(kch):
                        nc.tensor.matmul(
                            u_ps[ci][:, :cw], w_tiles[qt], e_tiles[qt][:, c0 : c0 + cw],
                            start=(qt == 0), stop=(qt == n_qt - 1),
                        )

                # u -> SBUF, broadcast to 128 partitions, t = u . h_v via DVE.
                # Mid-kernel dirs use a DRAM-roundtrip broadcast (no PSUM slot
                # contention); the last dir broadcasts via a rank-1 PE matmul
                # into the now-free score PSUM to cut the serial tail.
                u_sb = stats.tile([1, 1024], BF16, tag="u_sb", name=f"usb{bd}")
                for ci, (c0, cw) in enumerate(kch):
                    nc.vector.tensor_copy(out=u_sb[:, c0 : c0 + cw], in_=u_ps[ci][:, :cw])
                if last:
                    u_bc = psS.tile([128, 1024], F32, tag="ps_s", name=f"ubc{bd}")
                    for c0, cw in kch:
                        nc.tensor.matmul(
                            u_bc[:, c0 : c0 + cw],
                            ones_sb[:, :128],
                            u_sb[:, c0 : c0 + cw],
                            start=True,
                            stop=True,
                        )
                else:
                    u_dr = dramp.tile([1, kpad], BF16, tag="u_dr", name=f"udr{bd}")
                    nc.sync.dma_start(out=u_dr, in_=u_sb[:, :kpad])
                    u_bc = ubcp.tile([128, 1024], BF16, tag="u_bc", name=f"ubc{bd}")
                    nc.sync.dma_start(
                        out=u_bc[:, :kpad], in_=u_dr[0].partition_broadcast(128)
                    )
                for ht in range(HT):
                    prod = scrp.tile([128, 1024], BF16, tag="prod", name=f"prod{bd}_{ht}")
                    nc.vector.scalar_tensor_tensor(
                        out=prod[:, :kpad], in0=hb_k[:, 2 + ht, :kpad], scalar=1.0,
                        in1=u_bc[:, :kpad], op0=ALU.mult, op1=ALU.mult,
                        accum_out=t_f32[ht][:, bd : bd + 1],
                    )

            h_tiles = {}
            for seg in range(SEG):
                h_tiles[seg] = (
                    hpp.tile([128, 4, 1024], BF16, tag="hba", name=f"hba{seg}"),
                    hpp.tile([128, 4, 1024], BF16, tag="hbb", name=f"hbb{seg}"),
                    hpp.tile([128, 2, 1024], FP8, tag="h8a", name=f"h8a{seg}"),
                    hpp.tile([128, 2, 1024], FP8, tag="h8b", name=f"h8b{seg}"),
                )

            def mlp_gens(seg, act="dve"):
                hb_a, hb_b, h8_a, h8_b = h_tiles[seg]
                return (
                    mlp_units(seg, "a", xt_tiles[seg][0], hb_a, h8_a, act=act),
                    mlp_units(seg, "b", xt_tiles[seg][1], hb_b, h8_b, act=act),
                )

            # seg 0's MLP runs inline; each later segment's MLP is emitted as
            # filler inside the previous segment's attention score loops.
            g_a, g_b = mlp_gens(0, act="act")
            drain(g_a)
            drain(g_b)
            for seg in range(SEG):
                hb_a, hb_b, h8_a, h8_b = h_tiles[seg]
                nxt_a = nxt_b = None
                if seg + 1 < SEG:
                    load_xt(seg + 1, "a", xa_d, xt_tiles[seg + 1][0])
                    load_xt(seg + 1, "b", xb_d, xt_tiles[seg + 1][1])
                    nxt_a, nxt_b = mlp_gens(seg + 1)
                attention(seg, 0, hb_a, hb_b, h8_b, filler=nxt_a)
                attention(seg, 1, hb_b, hb_a, h8_a, last=(seg == SEG - 1), filler=nxt_b)
                drain(nxt_a)
                drain(nxt_b)

            # final projection, transposed so DRAM writes are contiguous:
            # o[ds, d] = sum_hi T[hi, ds] A_v[hi, d] + c_v[d]  (ds on partitions)
            t_bf = [tpool.tile([128, NDS], BF16, name=f"tb{ht}") for ht in range(HT)]
            for ht in range(HT):
                nc.vector.tensor_copy(out=t_bf[ht], in_=t_f32[ht])
            o_ps = psS.tile([128, D], F32, tag="ps_s", name="ops")
            for c0, cw in _chunks(D):
                for hi in range(HT):
                    nc.tensor.matmul(
                        o_ps[:NDS, c0 : c0 + cw],
                        t_bf[hi],
                        av_sb[:, hi * D + c0 : hi * D + c0 + cw],
                        start=(hi == 0),
                        stop=False,
                    )
                nc.tensor.matmul(
                    o_ps[:NDS, c0 : c0 + cw],
                    ones_sb[:, :NDS],
                    cv_sb[:, c0 : c0 + cw],
                    start=False,
                    stop=True,
                )
            o_sb = opool.tile([NDS, D], F32)
            nc.vector.tensor_copy(out=o_sb, in_=o_ps[:NDS, :])
            nc.sync.dma_start(out=o_d.rearrange("a s d -> (a s) d"), in_=o_sb)

    nc.compile()
    return nc


def _preprocess(inputs):
    """Host-side folding + sharding. Returns (sched, in_maps, perm)."""
    a = np.asarray(inputs["a"], dtype=np.float32)
    b = np.asarray(inputs["b"], dtype=np.float32)
    W1 = np.asarray(inputs["W1"], dtype=np.float32)
    b1 = np.asarray(inputs["b1"], dtype=np.float32)
    g = np.asarray(inputs["g"], dtype=np.float32)
    bt = np.asarray(inputs["bt"], dtype=np.float32)
    rm = np.asarray(inputs["rm"], dtype=np.float32)
    rv = np.asarray(inputs["rv"], dtype=np.float32)
    W2 = np.asarray(inputs["W2"], dtype=np.float32)
    b2 = np.asarray(inputs["b2"], dtype=np.float32)
    len_a = np.asarray(inputs["len_a"], dtype=np.int64)
    len_b = np.asarray(inputs["len_b"], dtype=np.int64)

    alpha = g / np.sqrt(rv + BN_EPS)
    beta = bt - rm * alpha
    A = W2 * alpha[:, :, None]  # [3, H, D]
    c = np.einsum("ph,phd->pd", beta, W2) + b2  # [3, D]
    # score scale (1/32) is NOT folded here: q~ is cast to fp8 on device and
    # the x32-larger values sit in e4m3's normal range; exp applies 1/SCALE.
    M = A[0] @ A[1].T  # [H, H]
    rk = A[1] @ c[0]  # [H]

    bf16 = ml_dtypes.bfloat16
    f8 = ml_dtypes.float8_e4m3
    # W1/b1 x8 keeps the small layer-1 weights out of fp8's subnormal range;
    # the kernel's Lrelu activation scale (1/8) undoes it after the matmul.
    # DoubleRow stationary layout: [p, pair, member, g*128+h] with
    # d = (2*pair+member)*128 + p.
    w1p = np.ascontiguousarray(
        (8.0 * W1).reshape(P, DT // 2, 2, 128, H // 128, 128)
        .transpose(3, 1, 2, 0, 4, 5)
        .reshape(128, DT // 2, 2, P * H)
        .astype(f8)
    )
    b1_bf = np.ascontiguousarray((8.0 * b1).astype(bf16))
    m_bf = np.ascontiguousarray(M.astype(bf16))
    rk_bf = np.ascontiguousarray(rk.astype(bf16))
    av_bf = np.ascontiguousarray(A[2].astype(bf16))
    cv_bf = np.ascontiguousarray(c[2].astype(bf16))

    # Segment -> (core, position): sort by score cost so each position's
    # cross-core max (which fixes the SPMD loop bounds) is small.
    order = np.argsort(-(len_a * len_b), kind="stable")
    perm = [[int(order[pos * N_CORES + cc]) for pos in range(SEG)] for cc in range(N_CORES)]

    sched = {}
    for pos in range(SEG):
        segs = [perm[cc][pos] for cc in range(N_CORES)]
        for dirn in range(2):
            lq = max((len_a if dirn == 0 else len_b)[s] for s in segs)
            lk = max((len_b if dirn == 0 else len_a)[s] for s in segs)
            sched[(dirn, pos)] = (
                _round_up(int(lq), 128) // 128,
                _round_up(int(lk), 128),
            )

    iota = np.arange(LA)
    in_maps = []
    for cc in range(N_CORES):
        segs = perm[cc]
        # [SEG, DT, 128, LA] feature-major fp8 layout (see _build_program)
        xa = np.ascontiguousarray(
            a[segs].reshape(SEG, LA, DT, 128).transpose(0, 2, 3, 1).astype(f8)
        )
        xb = np.ascontiguousarray(
            b[segs].reshape(SEG, LB, DT, 128).transpose(0, 2, 3, 1).astype(f8)
        )
        km = np.zeros((2, SEG, LA), dtype=np.float32)
        wb = np.zeros((2, SEG, LA), dtype=np.float32)
        for pos, s in enumerate(segs):
            for dirn in range(2):
                lq = int((len_a if dirn == 0 else len_b)[s])
                lk = int((len_b if dirn == 0 else len_a)[s])
                km[dirn, pos, :] = np.where(iota < lk, 0.0, NEG * SCALE)
                wb[dirn, pos, :] = np.where(iota < lq, 1.0 / lq, 0.0)
        in_maps.append(
            {
                "xa": xa,
                "xb": xb,
                "w1": w1p,
                "b1": b1_bf,
                "m": m_bf,
                "rk": rk_bf,
                "av": av_bf,
                "cv": cv_bf,
                "km": np.ascontiguousarray(km.astype(bf16)),
                "wb": np.ascontiguousarray(wb),
            }
        )
    return sched, in_maps, perm


def kernel(**inputs):
    global LAST_RESULTS
    from concourse.bass_utils import run_bass_kernel_spmd

    sched, in_maps, perm = _preprocess(inputs)
    key = tuple(sorted(sched.items()))
    if key not in _CACHE:
        _CACHE[key] = _build_program(sched)
    nc = _CACHE[key]

    res = run_bass_kernel_spmd(nc, in_maps, list(range(N_CORES)))
    LAST_RESULTS = res

    out = np.zeros((2, B, D), dtype=np.float32)
    for cc in range(N_CORES):
        o = res.results[cc]["o"]  # [2, SEG, D]
        for pos, s in enumerate(perm[cc]):
            out[0, s] = o[0, pos]
            out[1, s] = o[1, pos]
    return out


# revision 47
# speedup vs baseline: 1.3158x; 1.1727x over previous
"""Trainium2 Bass kernel for a 3-net MLP + masked mean-pooled cross-attention.

B=32 segments data-parallel across 8 NeuronCores (4 per core). The eval-mode
BatchNorm folds into the second MLP layer host-side (y_p = h_p @ A_p + c_p with
h_p the post-LeakyReLU hidden), which lets everything downstream contract
through H=256 instead of D=1024:

  * scores: s = q kT / 32 = h_q (A_q A_kT/32) h_kT + row-const + 1 (x) (rk.h_k)
    with M = A_q A_kT/32 [256,256] and rk = A_k c_q/32 precomputed host-side.
    Row-constant terms are invariant under the row softmax and are dropped;
    the rk term folds into q~ = h_q M + 1 (x) rk as a rank-1 PE update.
  * values: emb = u @ v = (u @ h_v) @ A_v + c_v (sum u = 1), so the [L, D]
    q/k/v tensors are never materialized and the second MLP layer collapses
    to one [256]-vector projection per (direction, segment).
  * max |score| ~ 4, so softmax needs no row-max subtraction; exp directly
    off the score PSUM with accumulated row-sums.
  * key masking is a rank-1 additive -1e6 update (ones (x) mask-row) into the
    score PSUM; exp underflows masked entries to exactly 0.
All matmul operands are bf16 with fp32 PSUM accumulation.
"""

import os
import sys

import numpy as np

for _p in ("/opt/trn_rl_repo", "/root/.axon_site/_ro/trn_rl_repo"):
    if os.path.isdir(_p) and _p not in sys.path:
        sys.path.insert(0, _p)

import ml_dtypes  # noqa: E402

B, LA, LB, D, H, P = 32, 1024, 1024, 1024, 256, 3
BN_EPS = 1e-5
SCALE = 32.0
N_CORES = 8
SEG = B // N_CORES
TOKBLK = 512
NEG = -1e6
DT = D // 128  # 8 d-tiles
HT = H // 128  # 2 h-tiles
NDS = 2 * SEG  # direction-segment slots per core

_CACHE = {}
LAST_RESULTS = None


def _round_up(x, m):
    return (x + m - 1) // m * m


def _chunks(n):
    out, c = [], 0
    while c < n:
        w = min(TOKBLK, n - c)
        out.append((c, w))
        c += w
    return out


def _build_program(sched):
    """sched[(dirn, pos)] = (n_qt, kpad): per segment-position loop structure,
    shared by all cores (SPMD). dirn 0: q from side a, k/v from b."""
    import concourse.bacc as bacc
    import concourse.mybir as mybir
    import concourse.tile as tile

    F32 = mybir.dt.float32
    BF16 = mybir.dt.bfloat16
    AF = mybir.ActivationFunctionType
    ALU = mybir.AluOpType

    nc = bacc.Bacc(
        "TRN2",
        target_bir_lowering=False,
        debug=False,
        enable_asserts=False,
        num_devices=N_CORES,
    )

    # x is pre-transposed host-side to [SEG, DT, 128, LA] so each side-segment
    # is one plain line-rate DMA (DMA_TRANSPOSE is ~2x slower and serializes).
    # x and W1 are fp8 e4m3 (layer-1 runs DoubleRow, 2 d-tiles per matmul);
    # W1/b1 are pre-scaled x8 host-side, undone by the Lrelu activation scale.
    FP8 = mybir.dt.float8e4
    NPAIR = DT // 2
    xa_d = nc.dram_tensor("xa", [SEG, DT, 128, LA], FP8, kind="ExternalInput").ap()
    xb_d = nc.dram_tensor("xb", [SEG, DT, 128, LB], FP8, kind="ExternalInput").ap()
    w1_d = nc.dram_tensor("w1", [128, NPAIR, 2, P * H], FP8, kind="ExternalInput").ap()
    b1_d = nc.dram_tensor("b1", [P, H], BF16, kind="ExternalInput").ap()
    m_d = nc.dram_tensor("m", [H, H], BF16, kind="ExternalInput").ap()
    rk_d = nc.dram_tensor("rk", [H], BF16, kind="ExternalInput").ap()
    av_d = nc.dram_tensor("av", [H, D], BF16, kind="ExternalInput").ap()
    cv_d = nc.dram_tensor("cv", [D], BF16, kind="ExternalInput").ap()
    km_d = nc.dram_tensor("km", [2, SEG, LA], BF16, kind="ExternalInput").ap()
    wb_d = nc.dram_tensor("wb", [2, SEG, LA], F32, kind="ExternalInput").ap()
    o_d = nc.dram_tensor("o", [2, SEG, D], F32, kind="ExternalOutput").ap()

    # per-position padded side lengths (side a / side b tokens needed)
    lpad = {}
    for pos in range(SEG):
        lpad[("a", pos)] = sched[(1, pos)][1]  # a is key side of dirn 1
        lpad[("b", pos)] = sched[(0, pos)][1]

    with tile.TileContext(nc) as tc:
        with (
            tc.tile_pool(name="consts", bufs=1) as consts,
            tc.tile_pool(name="xt", bufs=2) as xtp,
            tc.tile_pool(name="hp", bufs=2) as hpp,
            tc.tile_pool(name="qt", bufs=2) as qtp,
            tc.tile_pool(name="epool", bufs=9) as epool,
            tc.tile_pool(name="stats", bufs=10) as stats,
            tc.tile_pool(name="ubc", bufs=2) as ubcp,
            tc.tile_pool(name="scratch", bufs=2) as scrp,
            tc.tile_pool(name="tpool", bufs=1) as tpool,
            tc.tile_pool(name="opool", bufs=1) as opool,
            tc.tile_pool(name="psA", bufs=2, space="PSUM") as psA,
            tc.tile_pool(name="psS", bufs=2, space="PSUM") as psS,
            tc.tile_pool(name="psU", bufs=2, space="PSUM") as psU,
            tc.tile_pool(name="dramp", bufs=2, space="DRAM") as dramp,
        ):
            # ---- constants ----
            w1_sb = consts.tile([128, NPAIR, 2, P * H], FP8, name="w1sb")
            nc.sync.dma_start(out=w1_sb, in_=w1_d)
            b1_sb = consts.tile([1, P * H], BF16)
            nc.sync.dma_start(out=b1_sb, in_=b1_d.rearrange("p h -> (p h)").unsqueeze(0))
            ones_sb = consts.tile([1, TOKBLK], BF16)
            nc.vector.memset(ones_sb, 1.0)

            def load_xt(seg, side, x2d, xt):
                lp = lpad[(side, seg)]
                nc.sync.dma_start(
                    out=xt[:, :, :lp],
                    in_=x2d[seg].transpose([1, 0, 2])[:, :, :lp],
                )

            xt_tiles = {}
            for seg in range(SEG):
                xt_tiles[seg] = (
                    xtp.tile([128, DT, 1024], FP8, tag="xta", name=f"xta{seg}"),
                    xtp.tile([128, DT, 1024], FP8, tag="xtb", name=f"xtb{seg}"),
                )
            # seg 0's inputs ahead of the remaining consts, split per token
            # chunk: PE's first MLP group only needs w1/b1 + the first chunk.
            for side_i, (side, x2d) in enumerate((("a", xa_d), ("b", xb_d))):
                lp = lpad[(side, 0)]
                for c0, cw in _chunks(lp):
                    nc.sync.dma_start(
                        out=xt_tiles[0][side_i][:, :, c0 : c0 + cw],
                        in_=x2d[0].transpose([1, 0, 2])[:, :, c0 : c0 + cw],
                    )

            m_sb = consts.tile([128, HT * H], BF16)
            for hi in range(HT):
                nc.sync.dma_start(
                    out=m_sb[:, hi * H : (hi + 1) * H],
                    in_=m_d[hi * 128 : (hi + 1) * 128, :],
                )
            rk_sb = consts.tile([1, H], BF16)
            nc.sync.dma_start(out=rk_sb, in_=rk_d.unsqueeze(0))
            av_sb = consts.tile([128, HT * D], BF16)
            for hi in range(HT):
                nc.sync.dma_start(
                    out=av_sb[:, hi * D : (hi + 1) * D],
                    in_=av_d[hi * 128 : (hi + 1) * 128, :],
                )
            cv_sb = consts.tile([1, D], BF16)
            nc.sync.dma_start(out=cv_sb, in_=cv_d.unsqueeze(0))
            km_sb = consts.tile([1, 2 * SEG * LA], BF16)
            nc.sync.dma_start(out=km_sb, in_=km_d.rearrange("a s l -> (a s l)").unsqueeze(0))
            wb_sb = consts.tile([128, 2 * SEG * 8], F32)
            nc.sync.dma_start(out=wb_sb, in_=wb_d.rearrange("a s (t p) -> p (a s t)", p=128))
            t_f32 = [tpool.tile([128, NDS], F32, name=f"tf{ht}") for ht in range(HT)]

            def mlp(seg, side, xt, h_sb):
                """h_sb: [128, 6, 1024] bf16 feature-major hidden (6 = net*2+ht)."""
                lp = lpad[(side, seg)]
                for c0, cw in _chunks(lp):
                    for g in range(P * HT):
                        hp = psA.tile([128, TOKBLK], F32, tag="ps_a", name=f"hp{seg}{side}{g}{c0}")
                        for q in range(NPAIR):
                            nc.tensor.matmul(
                                hp[:, :cw],
                                w1_sb[:, q, :, g * 128 : (g + 1) * 128],
                                xt[:, 2 * q : 2 * q + 2, c0 : c0 + cw],
                                start=(q == 0),
                                stop=False,
                                perf_mode=mybir.MatmulPerfMode.DoubleRow,
                            )
                        nc.tensor.matmul(
                            hp[:, :cw],
                            b1_sb[:, g * 128 : (g + 1) * 128],
                            ones_sb[:, :cw],
                            start=False,
                            stop=True,
                        )
                        # LeakyReLU (slope 0.01 per PWP table); the 1/8 scale
                        # undoes the x8 pre-scaling of W1/b1. PSUM -> SBUF bf16.
                        nc.scalar.activation(
                            out=h_sb[:, g, c0 : c0 + cw], in_=hp[:, :cw],
                            func=AF.Lrelu, scale=0.125,
                        )

            def attention(seg, dirn, h_q, h_k, last=False):
                """h_q/h_k: [128, 6, 1024] bf16 tiles of the two sides. q-net
                groups 0..1 of h_q; k-net 2..3 and v-net 4..5 of h_k."""
                n_qt, kpad = sched[(dirn, seg)]
                lq = n_qt * 128
                kch = _chunks(kpad)
                bd = dirn * SEG + seg

                # q~ = h_q M + 1 (x) rk   [256, lq] feature-major bf16
                qt_sb = qtp.tile([128, HT, 1024], BF16, tag="qt", name=f"qt{bd}")
                for ho in range(HT):
                    for c0, cw in _chunks(lq):
                        qp = psA.tile([128, TOKBLK], F32, tag="ps_a", name=f"qp{bd}{ho}{c0}")
                        for hi in range(HT):
                            nc.tensor.matmul(
                                qp[:, :cw],
                                m_sb[:, hi * H + ho * 128 : hi * H + ho * 128 + 128],
                                h_q[:, hi, c0 : c0 + cw],
                                start=(hi == 0),
                                stop=False,
                            )
                        nc.tensor.matmul(
                            qp[:, :cw],
                            rk_sb[:, ho * 128 : (ho + 1) * 128],
                            ones_sb[:, :cw],
                            start=False,
                            stop=True,
                        )
                        nc.vector.tensor_copy(out=qt_sb[:, ho, c0 : c0 + cw], in_=qp[:, :cw])

                u_ps = [
                    psU.tile([1, TOKBLK], F32, tag="ps_u", name=f"u{bd}_{ci}")
                    for ci in range(len(kch))
                ]

                def softmax_u(qt, sp):
                    e = epool.tile([128, 1024], BF16, tag="e", name=f"e{bd}_{qt}")
                    z = stats.tile([128, 1], F32, tag="z", name=f"z{bd}_{qt}")
                    nc.scalar.activation(
                        out=e[:, :kpad], in_=sp[:, :kpad], func=AF.Exp, accum_out=z,
                    )
                    rz = stats.tile([128, 1], F32, tag="rz", name=f"rz{bd}_{qt}")
                    nc.vector.reciprocal(out=rz, in_=z)
                    w = stats.tile([128, 1], BF16, tag="w", name=f"w{bd}_{qt}")
                    nc.vector.tensor_tensor(
                        out=w, in0=wb_sb[:, bd * 8 + qt : bd * 8 + qt + 1], in1=rz,
                        op=ALU.mult,
                    )
                    for ci, (c0, cw) in enumerate(kch):
                        nc.tensor.matmul(
                            u_ps[ci][:, :cw], w, e[:, c0 : c0 + cw],
                            start=(qt == 0), stop=(qt == n_qt - 1),
                        )

                pend = None  # softmax of qt issued after scores of qt+1
                for qt in range(n_qt):
                    sp = psS.tile([128, 1024], F32, tag="ps_s", name=f"s{bd}_{qt}")
                    for hi in range(HT):
                        for c0, cw in kch:
                            nc.tensor.matmul(
                                sp[:, c0 : c0 + cw],
                                qt_sb[:, hi, qt * 128 : (qt + 1) * 128],
                                h_k[:, 2 + hi, c0 : c0 + cw],
                                start=(hi == 0),
                                stop=False,
                            )
                    for c0, cw in kch:
                        nc.tensor.matmul(
                            sp[:, c0 : c0 + cw],
                            ones_sb[:, :128],
                            km_sb[:, bd * LA + c0 : bd * LA + c0 + cw],
                            start=False,
                            stop=True,
                        )
                    if pend is not None:
                        softmax_u(*pend)
                    pend = (qt, sp)
                softmax_u(*pend)

                # u -> SBUF, broadcast to 128 partitions, t = u . h_v via DVE.
                # Mid-kernel dirs use a DRAM-roundtrip broadcast (no PSUM slot
                # contention); the last dir broadcasts via a rank-1 PE matmul
                # into the now-free score PSUM to cut the serial tail.
                u_sb = stats.tile([1, 1024], BF16, tag="u_sb", name=f"usb{bd}")
                for ci, (c0, cw) in enumerate(kch):
                    nc.vector.tensor_copy(out=u_sb[:, c0 : c0 + cw], in_=u_ps[ci][:, :cw])
                if last:
                    u_bc = psS.tile([128, 1024], F32, tag="ps_s", name=f"ubc{bd}")
                    for c0, cw in kch:
                        nc.tensor.matmul(
                            u_bc[:, c0 : c0 + cw],
                            ones_sb[:, :128],
                            u_sb[:, c0 : c0 + cw],
                            start=True,
                            stop=True,
                        )
                else:
                    u_dr = dramp.tile([1, kpad], BF16, tag="u_dr", name=f"udr{bd}")
                    nc.sync.dma_start(out=u_dr, in_=u_sb[:, :kpad])
                    u_bc = ubcp.tile([128, 1024], BF16, tag="u_bc", name=f"ubc{bd}")
                    nc.sync.dma_start(
                        out=u_bc[:, :kpad], in_=u_dr[0].partition_broadcast(128)
                    )
                for ht in range(HT):
                    prod = scrp.tile([128, 1024], BF16, tag="prod", name=f"prod{bd}_{ht}")
                    nc.vector.scalar_tensor_tensor(
                        out=prod[:, :kpad], in0=h_k[:, 4 + ht, :kpad], scalar=1.0,
                        in1=u_bc[:, :kpad], op0=ALU.mult, op1=ALU.mult,
                        accum_out=t_f32[ht][:, bd : bd + 1],
                    )

            for seg in range(SEG):
                if seg > 0:
                    load_xt(seg, "a", xa_d, xt_tiles[seg][0])
                    load_xt(seg, "b", xb_d, xt_tiles[seg][1])
                h_a = hpp.tile([128, P * HT, 1024], BF16, tag="ha", name=f"ha{seg}")
                h_b = hpp.tile([128, P * HT, 1024], BF16, tag="hb", name=f"hb{seg}")
                mlp(seg, "a", xt_tiles[seg][0], h_a)
                mlp(seg, "b", xt_tiles[seg][1], h_b)
                attention(seg, 0, h_a, h_b)
                attention(seg, 1, h_b, h_a, last=(seg == SEG - 1))

            # final projection, transposed so DRAM writes are contiguous:
            # o[ds, d] = sum_hi T[hi, ds] A_v[hi, d] + c_v[d]  (ds on partitions)
            t_bf = [tpool.tile([128, NDS], BF16, name=f"tb{ht}") for ht in range(HT)]
            for ht in range(HT):
                nc.vector.tensor_copy(out=t_bf[ht], in_=t_f32[ht])
            o_ps = psS.tile([128, D], F32, tag="ps_s", name="ops")
            for c0, cw in _chunks(D):
                for hi in range(HT):
                    nc.tensor.matmul(
                        o_ps[:NDS, c0 : c0 + cw],
                        t_bf[hi],
                        av_sb[:, hi * D + c0 : hi * D + c0 + cw],
                        start=(hi == 0),
                        stop=False,
                    )
                nc.tensor.matmul(
                    o_ps[:NDS, c0 : c0 + cw],
                    ones_sb[:, :NDS],
                    cv_sb[:, c0 : c0 + cw],
                    start=False,
                    stop=True,
                )
            o_sb = opool.tile([NDS, D], F32)
            nc.vector.tensor_copy(out=o_sb, in_=o_ps[:NDS, :])
            nc.sync.dma_start(out=o_d.rearrange("a s d -> (a s) d"), in_=o_sb)

    nc.compile()
    return nc


def _preprocess(inputs):
    """Host-side folding + sharding. Returns (sched, in_maps, perm)."""
    a = np.asarray(inputs["a"], dtype=np.float32)
    b = np.asarray(inputs["b"], dtype=np.float32)
    W1 = np.asarray(inputs["W1"], dtype=np.float32)
    b1 = np.asarray(inputs["b1"], dtype=np.float32)
    g = np.asarray(inputs["g"], dtype=np.float32)
    bt = np.asarray(inputs["bt"], dtype=np.float32)
    rm = np.asarray(inputs["rm"], dtype=np.float32)
    rv = np.asarray(inputs["rv"], dtype=np.float32)
    W2 = np.asarray(inputs["W2"], dtype=np.float32)
    b2 = np.asarray(inputs["b2"], dtype=np.float32)
    len_a = np.asarray(inputs["len_a"], dtype=np.int64)
    len_b = np.asarray(inputs["len_b"], dtype=np.int64)

    alpha = g / np.sqrt(rv + BN_EPS)
    beta = bt - rm * alpha
    A = W2 * alpha[:, :, None]  # [3, H, D]
    c = np.einsum("ph,phd->pd", beta, W2) + b2  # [3, D]
    M = A[0] @ A[1].T / SCALE  # [H, H]
    rk = A[1] @ c[0] / SCALE  # [H]

    bf16 = ml_dtypes.bfloat16
    f8 = ml_dtypes.float8_e4m3
    # W1/b1 x8 keeps the small layer-1 weights out of fp8's subnormal range;
    # the kernel's Lrelu activation scale (1/8) undoes it after the matmul.
    # DoubleRow stationary layout: [p, pair, member, g*128+h] with
    # d = (2*pair+member)*128 + p.
    w1p = np.ascontiguousarray(
        (8.0 * W1).reshape(P, DT // 2, 2, 128, H // 128, 128)
        .transpose(3, 1, 2, 0, 4, 5)
        .reshape(128, DT // 2, 2, P * H)
        .astype(f8)
    )
    b1_bf = np.ascontiguousarray((8.0 * b1).astype(bf16))
    m_bf = np.ascontiguousarray(M.astype(bf16))
    rk_bf = np.ascontiguousarray(rk.astype(bf16))
    av_bf = np.ascontiguousarray(A[2].astype(bf16))
    cv_bf = np.ascontiguousarray(c[2].astype(bf16))

    # Segment -> (core, position): sort by score cost so each position's
    # cross-core max (which fixes the SPMD loop bounds) is small.
    order = np.argsort(-(len_a * len_b), kind="stable")
    perm = [[int(order[pos * N_CORES + cc]) for pos in range(SEG)] for cc in range(N_CORES)]

    sched = {}
    for pos in range(SEG):
        segs = [perm[cc][pos] for cc in range(N_CORES)]
        for dirn in range(2):
            lq = max((len_a if dirn == 0 else len_b)[s] for s in segs)
            lk = max((len_b if dirn == 0 else len_a)[s] for s in segs)
            sched[(dirn, pos)] = (
                _round_up(int(lq), 128) // 128,
                _round_up(int(lk), 128),
            )

    iota = np.arange(LA)
    in_maps = []
    for cc in range(N_CORES):
        segs = perm[cc]
        # [SEG, DT, 128, LA] feature-major fp8 layout (see _build_program)
        xa = np.ascontiguousarray(
            a[segs].reshape(SEG, LA, DT, 128).transpose(0, 2, 3, 1).astype(f8)
        )
        xb = np.ascontiguousarray(
            b[segs].reshape(SEG, LB, DT, 128).transpose(0, 2, 3, 1).astype(f8)
        )
        km = np.zeros((2, SEG, LA), dtype=np.float32)
        wb = np.zeros((2, SEG, LA), dtype=np.float32)
        for pos, s in enumerate(segs):
            for dirn in range(2):
                lq = int((len_a if dirn == 0 else len_b)[s])
                lk = int((len_b if dirn == 0 else len_a)[s])
                km[dirn, pos, :] = np.where(iota < lk, 0.0, NEG)
                wb[dirn, pos, :] = np.where(iota < lq, 1.0 / lq, 0.0)
        in_maps.append(
            {
                "xa": xa,
                "xb": xb,
                "w1": w1p,
                "b1": b1_bf,
                "m": m_bf,
                "rk": rk_bf,
                "av": av_bf,
                "cv": cv_bf,
                "km": np.ascontiguousarray(km.astype(bf16)),
                "wb": np.ascontiguousarray(wb),
            }
        )
    return sched, in_maps, perm


def kernel(**inputs):
    global LAST_RESULTS
    from concourse.bass_utils import run_bass_kernel_spmd

    sched, in_maps, perm = _preprocess(inputs)
    key = tuple(sorted(sched.items()))
    if key not in _CACHE:
        _CACHE[key] = _build_program(sched)
    nc = _CACHE[key]

    res = run_bass_kernel_spmd(nc, in_maps, list(range(N_CORES)))
    LAST_RESULTS = res

    out = np.zeros((2, B, D), dtype=np.float32)
    for cc in range(N_CORES):
        o = res.results[cc]["o"]  # [2, SEG, D]
        for pos, s in enumerate(perm[cc]):
            out[0, s] = o[0, pos]
            out[1, s] = o[1, pos]
    return out


# revision 48
# speedup vs baseline: 1.3662x; 1.0384x over previous
"""Trainium2 Bass kernel for a 3-net MLP + masked mean-pooled cross-attention.

B=32 segments data-parallel across 8 NeuronCores (4 per core). The eval-mode
BatchNorm folds into the second MLP layer host-side (y_p = h_p @ A_p + c_p with
h_p the post-LeakyReLU hidden), which lets everything downstream contract
through H=256 instead of D=1024:

  * scores: s = q kT / 32 = h_q (A_q A_kT/32) h_kT + row-const + 1 (x) (rk.h_k)
    with M = A_q A_kT/32 [256,256] and rk = A_k c_q/32 precomputed host-side.
    Row-constant terms are invariant under the row softmax and are dropped;
    the rk term folds into q~ = h_q M + 1 (x) rk as a rank-1 PE update.
  * values: emb = u @ v = (u @ h_v) @ A_v + c_v (sum u = 1), so the [L, D]
    q/k/v tensors are never materialized and the second MLP layer collapses
    to one [256]-vector projection per (direction, segment).
  * max |score| ~ 4, so softmax needs no row-max subtraction; exp directly
    off the score PSUM with accumulated row-sums.
  * key masking is a rank-1 additive -1e6 update (ones (x) mask-row) into the
    score PSUM; exp underflows masked entries to exactly 0.
All matmul operands are bf16 with fp32 PSUM accumulation.
"""

import os
import sys

import numpy as np

for _p in ("/opt/trn_rl_repo", "/root/.axon_site/_ro/trn_rl_repo"):
    if os.path.isdir(_p) and _p not in sys.path:
        sys.path.insert(0, _p)

import ml_dtypes  # noqa: E402

B, LA, LB, D, H, P = 32, 1024, 1024, 1024, 256, 3
BN_EPS = 1e-5
SCALE = 32.0
N_CORES = 8
SEG = B // N_CORES
TOKBLK = 512
NEG = -1e6
DT = D // 128  # 8 d-tiles
HT = H // 128  # 2 h-tiles
NDS = 2 * SEG  # direction-segment slots per core

_CACHE = {}
LAST_RESULTS = None


def _round_up(x, m):
    return (x + m - 1) // m * m


def _chunks(n):
    out, c = [], 0
    while c < n:
        w = min(TOKBLK, n - c)
        out.append((c, w))
        c += w
    return out


def _build_program(sched):
    """sched[(dirn, pos)] = (n_qt, kpad): per segment-position loop structure,
    shared by all cores (SPMD). dirn 0: q from side a, k/v from b."""
    import concourse.bacc as bacc
    import concourse.mybir as mybir
    import concourse.tile as tile

    F32 = mybir.dt.float32
    BF16 = mybir.dt.bfloat16
    AF = mybir.ActivationFunctionType
    ALU = mybir.AluOpType

    nc = bacc.Bacc(
        "TRN2",
        target_bir_lowering=False,
        debug=False,
        enable_asserts=False,
        num_devices=N_CORES,
    )

    # x is pre-transposed host-side to [SEG, DT, 128, LA] so each side-segment
    # is one plain line-rate DMA (DMA_TRANSPOSE is ~2x slower and serializes).
    # x and W1 are fp8 e4m3 (layer-1 runs DoubleRow, 2 d-tiles per matmul);
    # W1/b1 are pre-scaled x8 host-side, undone by the Lrelu activation scale.
    FP8 = mybir.dt.float8e4
    NPAIR = DT // 2
    xa_d = nc.dram_tensor("xa", [SEG, DT, 128, LA], FP8, kind="ExternalInput").ap()
    xb_d = nc.dram_tensor("xb", [SEG, DT, 128, LB], FP8, kind="ExternalInput").ap()
    w1_d = nc.dram_tensor("w1", [128, NPAIR, 2, P * H], FP8, kind="ExternalInput").ap()
    b1_d = nc.dram_tensor("b1", [P, H], BF16, kind="ExternalInput").ap()
    m_d = nc.dram_tensor("m", [H, H], BF16, kind="ExternalInput").ap()
    rk_d = nc.dram_tensor("rk", [H], BF16, kind="ExternalInput").ap()
    av_d = nc.dram_tensor("av", [H, D], BF16, kind="ExternalInput").ap()
    cv_d = nc.dram_tensor("cv", [D], BF16, kind="ExternalInput").ap()
    km_d = nc.dram_tensor("km", [2, SEG, LA], BF16, kind="ExternalInput").ap()
    wb_d = nc.dram_tensor("wb", [2, SEG, LA], F32, kind="ExternalInput").ap()
    o_d = nc.dram_tensor("o", [2, SEG, D], F32, kind="ExternalOutput").ap()

    # per-position padded side lengths (side a / side b tokens needed)
    lpad = {}
    for pos in range(SEG):
        lpad[("a", pos)] = sched[(1, pos)][1]  # a is key side of dirn 1
        lpad[("b", pos)] = sched[(0, pos)][1]

    with tile.TileContext(nc) as tc:
        with (
            tc.tile_pool(name="consts", bufs=1) as consts,
            tc.tile_pool(name="xt", bufs=2) as xtp,
            tc.tile_pool(name="hp", bufs=2) as hpp,
            tc.tile_pool(name="qt", bufs=2) as qtp,
            tc.tile_pool(name="epool", bufs=9) as epool,
            tc.tile_pool(name="stats", bufs=10) as stats,
            tc.tile_pool(name="ubc", bufs=2) as ubcp,
            tc.tile_pool(name="scratch", bufs=2) as scrp,
            tc.tile_pool(name="tpool", bufs=1) as tpool,
            tc.tile_pool(name="opool", bufs=1) as opool,
            tc.tile_pool(name="psA", bufs=2, space="PSUM") as psA,
            tc.tile_pool(name="psS", bufs=2, space="PSUM") as psS,
            tc.tile_pool(name="psU", bufs=2, space="PSUM") as psU,
            tc.tile_pool(name="dramp", bufs=2, space="DRAM") as dramp,
        ):
            # ---- constants ----
            w1_sb = consts.tile([128, NPAIR, 2, P * H], FP8, name="w1sb")
            nc.sync.dma_start(out=w1_sb, in_=w1_d)
            b1_sb = consts.tile([1, P * H], BF16)
            nc.sync.dma_start(out=b1_sb, in_=b1_d.rearrange("p h -> (p h)").unsqueeze(0))
            ones_sb = consts.tile([1, TOKBLK], BF16)
            nc.vector.memset(ones_sb, 1.0)

            def load_xt(seg, side, x2d, xt):
                lp = lpad[(side, seg)]
                nc.sync.dma_start(
                    out=xt[:, :, :lp],
                    in_=x2d[seg].transpose([1, 0, 2])[:, :, :lp],
                )

            xt_tiles = {}
            for seg in range(SEG):
                xt_tiles[seg] = (
                    xtp.tile([128, DT, 1024], FP8, tag="xta", name=f"xta{seg}"),
                    xtp.tile([128, DT, 1024], FP8, tag="xtb", name=f"xtb{seg}"),
                )
            # seg 0's inputs ahead of the remaining consts, split per token
            # chunk: PE's first MLP group only needs w1/b1 + the first chunk.
            for side_i, (side, x2d) in enumerate((("a", xa_d), ("b", xb_d))):
                lp = lpad[(side, 0)]
                for c0, cw in _chunks(lp):
                    nc.sync.dma_start(
                        out=xt_tiles[0][side_i][:, :, c0 : c0 + cw],
                        in_=x2d[0].transpose([1, 0, 2])[:, :, c0 : c0 + cw],
                    )

            m_sb = consts.tile([128, HT * H], BF16)
            for hi in range(HT):
                nc.sync.dma_start(
                    out=m_sb[:, hi * H : (hi + 1) * H],
                    in_=m_d[hi * 128 : (hi + 1) * 128, :],
                )
            rk_sb = consts.tile([1, H], BF16)
            nc.sync.dma_start(out=rk_sb, in_=rk_d.unsqueeze(0))
            av_sb = consts.tile([128, HT * D], BF16)
            for hi in range(HT):
                nc.sync.dma_start(
                    out=av_sb[:, hi * D : (hi + 1) * D],
                    in_=av_d[hi * 128 : (hi + 1) * 128, :],
                )
            cv_sb = consts.tile([1, D], BF16)
            nc.sync.dma_start(out=cv_sb, in_=cv_d.unsqueeze(0))
            km_sb = consts.tile([1, 2 * SEG * LA], BF16)
            nc.sync.dma_start(out=km_sb, in_=km_d.rearrange("a s l -> (a s l)").unsqueeze(0))
            wb_sb = consts.tile([128, 2 * SEG * 8], F32)
            nc.sync.dma_start(out=wb_sb, in_=wb_d.rearrange("a s (t p) -> p (a s t)", p=128))
            t_f32 = [tpool.tile([128, NDS], F32, name=f"tf{ht}") for ht in range(HT)]

            def mlp(seg, side, xt, h_sb):
                """h_sb: [128, 6, 1024] bf16 feature-major hidden (6 = net*2+ht)."""
                lp = lpad[(side, seg)]
                for c0, cw in _chunks(lp):
                    for g in range(P * HT):
                        hp = psA.tile([128, TOKBLK], F32, tag="ps_a", name=f"hp{seg}{side}{g}{c0}")
                        for q in range(NPAIR):
                            nc.tensor.matmul(
                                hp[:, :cw],
                                w1_sb[:, q, :, g * 128 : (g + 1) * 128],
                                xt[:, 2 * q : 2 * q + 2, c0 : c0 + cw],
                                start=(q == 0),
                                stop=False,
                                perf_mode=mybir.MatmulPerfMode.DoubleRow,
                            )
                        nc.tensor.matmul(
                            hp[:, :cw],
                            b1_sb[:, g * 128 : (g + 1) * 128],
                            ones_sb[:, :cw],
                            start=False,
                            stop=True,
                        )
                        # LeakyReLU (slope 0.01 per PWP table); the 1/8 scale
                        # undoes the x8 pre-scaling of W1/b1. PSUM -> SBUF bf16.
                        nc.scalar.activation(
                            out=h_sb[:, g, c0 : c0 + cw], in_=hp[:, :cw],
                            func=AF.Lrelu, scale=0.125,
                        )

            def attention(seg, dirn, h_q, h_k, last=False):
                """h_q/h_k: [128, 6, 1024] bf16 tiles of the two sides. q-net
                groups 0..1 of h_q; k-net 2..3 and v-net 4..5 of h_k."""
                n_qt, kpad = sched[(dirn, seg)]
                lq = n_qt * 128
                kch = _chunks(kpad)
                bd = dirn * SEG + seg

                # q~ = h_q M + 1 (x) rk   [256, lq] feature-major bf16
                qt_sb = qtp.tile([128, HT, 1024], BF16, tag="qt", name=f"qt{bd}")
                for ho in range(HT):
                    for c0, cw in _chunks(lq):
                        qp = psA.tile([128, TOKBLK], F32, tag="ps_a", name=f"qp{bd}{ho}{c0}")
                        for hi in range(HT):
                            nc.tensor.matmul(
                                qp[:, :cw],
                                m_sb[:, hi * H + ho * 128 : hi * H + ho * 128 + 128],
                                h_q[:, hi, c0 : c0 + cw],
                                start=(hi == 0),
                                stop=False,
                            )
                        nc.tensor.matmul(
                            qp[:, :cw],
                            rk_sb[:, ho * 128 : (ho + 1) * 128],
                            ones_sb[:, :cw],
                            start=False,
                            stop=True,
                        )
                        nc.vector.tensor_copy(out=qt_sb[:, ho, c0 : c0 + cw], in_=qp[:, :cw])

                u_ps = [
                    psU.tile([1, TOKBLK], F32, tag="ps_u", name=f"u{bd}_{ci}")
                    for ci in range(len(kch))
                ]

                def softmax_u(qt, sp):
                    e = epool.tile([128, 1024], BF16, tag="e", name=f"e{bd}_{qt}")
                    z = stats.tile([128, 1], F32, tag="z", name=f"z{bd}_{qt}")
                    nc.scalar.activation(
                        out=e[:, :kpad], in_=sp[:, :kpad], func=AF.Exp, accum_out=z,
                    )
                    rz = stats.tile([128, 1], F32, tag="rz", name=f"rz{bd}_{qt}")
                    nc.vector.reciprocal(out=rz, in_=z)
                    w = stats.tile([128, 1], BF16, tag="w", name=f"w{bd}_{qt}")
                    nc.vector.tensor_tensor(
                        out=w, in0=wb_sb[:, bd * 8 + qt : bd * 8 + qt + 1], in1=rz,
                        op=ALU.mult,
                    )
                    for ci, (c0, cw) in enumerate(kch):
                        nc.tensor.matmul(
                            u_ps[ci][:, :cw], w, e[:, c0 : c0 + cw],
                            start=(qt == 0), stop=(qt == n_qt - 1),
                        )

                pend = None  # softmax of qt issued after scores of qt+1
                for qt in range(n_qt):
                    sp = psS.tile([128, 1024], F32, tag="ps_s", name=f"s{bd}_{qt}")
                    for hi in range(HT):
                        for c0, cw in kch:
                            nc.tensor.matmul(
                                sp[:, c0 : c0 + cw],
                                qt_sb[:, hi, qt * 128 : (qt + 1) * 128],
                                h_k[:, 2 + hi, c0 : c0 + cw],
                                start=(hi == 0),
                                stop=False,
                            )
                    for c0, cw in kch:
                        nc.tensor.matmul(
                            sp[:, c0 : c0 + cw],
                            ones_sb[:, :128],
                            km_sb[:, bd * LA + c0 : bd * LA + c0 + cw],
                            start=False,
                            stop=True,
                        )
                    if pend is not None:
                        softmax_u(*pend)
                    pend = (qt, sp)
                softmax_u(*pend)

                # u -> SBUF, broadcast to 128 partitions, t = u . h_v via DVE.
                # Mid-kernel dirs use a DRAM-roundtrip broadcast (no PSUM slot
                # contention); the last dir broadcasts via a rank-1 PE matmul
                # into the now-free score PSUM to cut the serial tail.
                u_sb = stats.tile([1, 1024], BF16, tag="u_sb", name=f"usb{bd}")
                for ci, (c0, cw) in enumerate(kch):
                    nc.vector.tensor_copy(out=u_sb[:, c0 : c0 + cw], in_=u_ps[ci][:, :cw])
                if last:
                    u_bc = psS.tile([128, 1024], F32, tag="ps_s", name=f"ubc{bd}")
                    for c0, cw in kch:
                        nc.tensor.matmul(
                            u_bc[:, c0 : c0 + cw],
                            ones_sb[:, :128],
                            u_sb[:, c0 : c0 + cw],
                            start=True,
                            stop=True,
                        )
                else:
                    u_dr = dramp.tile([1, kpad], BF16, tag="u_dr", name=f"udr{bd}")
                    nc.sync.dma_start(out=u_dr, in_=u_sb[:, :kpad])
                    u_bc = ubcp.tile([128, 1024], BF16, tag="u_bc", name=f"ubc{bd}")
                    nc.sync.dma_start(
                        out=u_bc[:, :kpad], in_=u_dr[0].partition_broadcast(128)
                    )
                for ht in range(HT):
                    prod = scrp.tile([128, 1024], BF16, tag="prod", name=f"prod{bd}_{ht}")
                    nc.vector.scalar_tensor_tensor(
                        out=prod[:, :kpad], in0=h_k[:, 4 + ht, :kpad], scalar=1.0,
                        in1=u_bc[:, :kpad], op0=ALU.mult, op1=ALU.mult,
                        accum_out=t_f32[ht][:, bd : bd + 1],
                    )

            for seg in range(SEG):
                if seg > 0:
                    load_xt(seg, "a", xa_d, xt_tiles[seg][0])
                    load_xt(seg, "b", xb_d, xt_tiles[seg][1])
                h_a = hpp.tile([128, P * HT, 1024], BF16, tag="ha", name=f"ha{seg}")
                h_b = hpp.tile([128, P * HT, 1024], BF16, tag="hb", name=f"hb{seg}")
                mlp(seg, "a", xt_tiles[seg][0], h_a)
                mlp(seg, "b", xt_tiles[seg][1], h_b)
                attention(seg, 0, h_a, h_b)
                attention(seg, 1, h_b, h_a, last=(seg == SEG - 1))

            # final projection, transposed so DRAM writes are contiguous:
            # o[ds, d] = sum_hi T[hi, ds] A_v[hi, d] + c_v[d]  (ds on partitions)
            t_bf = [tpool.tile([128, NDS], BF16, name=f"tb{ht}") for ht in range(HT)]
            for ht in range(HT):
                nc.vector.tensor_copy(out=t_bf[ht], in_=t_f32[ht])
            o_ps = psS.tile([128, D], F32, tag="ps_s", name="ops")
            for c0, cw in _chunks(D):
                for hi in range(HT):
                    nc.tensor.matmul(
                        o_ps[:NDS, c0 : c0 + cw],
                        t_bf[hi],
                        av_sb[:, hi * D + c0 : hi * D + c0 + cw],
                        start=(hi == 0),
                        stop=False,
                    )
                nc.tensor.matmul(
                    o_ps[:NDS, c0 : c0 + cw],
                    ones_sb[:, :NDS],
                    cv_sb[:, c0 : c0 + cw],
                    start=False,
                    stop=True,
                )
            o_sb = opool.tile([NDS, D], F32)
            nc.vector.tensor_copy(out=o_sb, in_=o_ps[:NDS, :])
            nc.sync.dma_start(out=o_d.rearrange("a s d -> (a s) d"), in_=o_sb)

    nc.compile()
    return nc


def _preprocess(inputs):
    """Host-side folding + sharding. Returns (sched, in_maps, perm)."""
    a = np.asarray(inputs["a"], dtype=np.float32)
    b = np.asarray(inputs["b"], dtype=np.float32)
    W1 = np.asarray(inputs["W1"], dtype=np.float32)
    b1 = np.asarray(inputs["b1"], dtype=np.float32)
    g = np.asarray(inputs["g"], dtype=np.float32)
    bt = np.asarray(inputs["bt"], dtype=np.float32)
    rm = np.asarray(inputs["rm"], dtype=np.float32)
    rv = np.asarray(inputs["rv"], dtype=np.float32)
    W2 = np.asarray(inputs["W2"], dtype=np.float32)
    b2 = np.asarray(inputs["b2"], dtype=np.float32)
    len_a = np.asarray(inputs["len_a"], dtype=np.int64)
    len_b = np.asarray(inputs["len_b"], dtype=np.int64)

    alpha = g / np.sqrt(rv + BN_EPS)
    beta = bt - rm * alpha
    A = W2 * alpha[:, :, None]  # [3, H, D]
    c = np.einsum("ph,phd->pd", beta, W2) + b2  # [3, D]
    M = A[0] @ A[1].T / SCALE  # [H, H]
    rk = A[1] @ c[0] / SCALE  # [H]

    bf16 = ml_dtypes.bfloat16
    f8 = ml_dtypes.float8_e4m3
    # W1/b1 x8 keeps the small layer-1 weights out of fp8's subnormal range;
    # the kernel's Lrelu activation scale (1/8) undoes it after the matmul.
    # DoubleRow stationary layout: [p, pair, member, g*128+h] with
    # d = (2*pair+member)*128 + p.
    w1p = np.ascontiguousarray(
        (8.0 * W1).reshape(P, DT // 2, 2, 128, H // 128, 128)
        .transpose(3, 1, 2, 0, 4, 5)
        .reshape(128, DT // 2, 2, P * H)
        .astype(f8)
    )
    b1_bf = np.ascontiguousarray((8.0 * b1).astype(bf16))
    m_bf = np.ascontiguousarray(M.astype(bf16))
    rk_bf = np.ascontiguousarray(rk.astype(bf16))
    av_bf = np.ascontiguousarray(A[2].astype(bf16))
    cv_bf = np.ascontiguousarray(c[2].astype(bf16))

    # Segment -> (core, position) assignment. The SPMD loop bounds at each
    # position are the cross-core max lengths, so pick the grouping that
    # minimizes a PE-cycle cost model (linear MLP/q~ term + score product
    # term), starting from a few sort orders + deterministic swap hill-climb.
    def _cost_of(perm_):
        tot = 0.0
        for pos in range(SEG):
            segs_ = [perm_[cc][pos] for cc in range(N_CORES)]
            lpa = _round_up(int(max(len_a[s] for s in segs_)), 128)
            lpb = _round_up(int(max(len_b[s] for s in segs_)), 128)
            tot += 32.0 * (lpa + lpb) + 0.047 * lpa * lpb
        return tot

    def _blocks(order_):
        return [
            [int(order_[pos * N_CORES + cc]) for pos in range(SEG)]
            for cc in range(N_CORES)
        ]

    cands = [
        np.argsort(-(len_a * len_b), kind="stable"),
        np.argsort(-(len_a + len_b), kind="stable"),
        np.argsort(-len_a, kind="stable"),
        np.argsort(-len_b, kind="stable"),
    ]
    perm = min((_blocks(o) for o in cands), key=_cost_of)
    perm = [list(r) for r in perm]
    cost = _cost_of(perm)
    improved = True
    while improved:
        improved = False
        for c1 in range(N_CORES):
            for p1 in range(SEG):
                for c2 in range(N_CORES):
                    for p2 in range(p1 + 1, SEG):
                        perm[c1][p1], perm[c2][p2] = perm[c2][p2], perm[c1][p1]
                        c_new = _cost_of(perm)
                        if c_new < cost - 1e-9:
                            cost = c_new
                            improved = True
                        else:
                            perm[c1][p1], perm[c2][p2] = perm[c2][p2], perm[c1][p1]

    sched = {}
    for pos in range(SEG):
        segs = [perm[cc][pos] for cc in range(N_CORES)]
        for dirn in range(2):
            lq = max((len_a if dirn == 0 else len_b)[s] for s in segs)
            lk = max((len_b if dirn == 0 else len_a)[s] for s in segs)
            sched[(dirn, pos)] = (
                _round_up(int(lq), 128) // 128,
                _round_up(int(lk), 128),
            )

    iota = np.arange(LA)
    in_maps = []
    for cc in range(N_CORES):
        segs = perm[cc]
        # [SEG, DT, 128, LA] feature-major fp8 layout (see _build_program)
        xa = np.ascontiguousarray(
            a[segs].reshape(SEG, LA, DT, 128).transpose(0, 2, 3, 1).astype(f8)
        )
        xb = np.ascontiguousarray(
            b[segs].reshape(SEG, LB, DT, 128).transpose(0, 2, 3, 1).astype(f8)
        )
        km = np.zeros((2, SEG, LA), dtype=np.float32)
        wb = np.zeros((2, SEG, LA), dtype=np.float32)
        for pos, s in enumerate(segs):
            for dirn in range(2):
                lq = int((len_a if dirn == 0 else len_b)[s])
                lk = int((len_b if dirn == 0 else len_a)[s])
                km[dirn, pos, :] = np.where(iota < lk, 0.0, NEG)
                wb[dirn, pos, :] = np.where(iota < lq, 1.0 / lq, 0.0)
        in_maps.append(
            {
                "xa": xa,
                "xb": xb,
                "w1": w1p,
                "b1": b1_bf,
                "m": m_bf,
                "rk": rk_bf,
                "av": av_bf,
                "cv": cv_bf,
                "km": np.ascontiguousarray(km.astype(bf16)),
                "wb": np.ascontiguousarray(wb),
            }
        )
    return sched, in_maps, perm


def kernel(**inputs):
    global LAST_RESULTS
    from concourse.bass_utils import run_bass_kernel_spmd

    sched, in_maps, perm = _preprocess(inputs)
    key = tuple(sorted(sched.items()))
    if key not in _CACHE:
        _CACHE[key] = _build_program(sched)
    nc = _CACHE[key]

    res = run_bass_kernel_spmd(nc, in_maps, list(range(N_CORES)))
    LAST_RESULTS = res

    out = np.zeros((2, B, D), dtype=np.float32)
    for cc in range(N_CORES):
        o = res.results[cc]["o"]  # [2, SEG, D]
        for pos, s in enumerate(perm[cc]):
            out[0, s] = o[0, pos]
            out[1, s] = o[1, pos]
    return out
